# revision 11
# baseline (speedup 1.0000x reference)
"""Trainium2 Bass kernel: 3x3 SAME conv (NCHW/OIHW) + bias.

Full problem: inp (32,128,56,56) f32, kernel (256,128,3,3) f32, bias (256,) f32
-> out (32,256,56,56) f32.

Strategy: data-parallel over batch across 8 cores (4 images/core). Host-side
prep inside kernel(): zero-pad images to [128,58,58], transpose weights to
[C,O,9], reshape bias to [128,2] — every device DMA is contiguous. Per image,
implicit GEMM: contraction K = C_in = 128 on the partition dim, M = 128 output
channels per tile (2 tiles), moving N = 448 spatial pixels (8 output rows x 56
cols via a strided AP over the padded SBUF image).

DT_MODE selects the PE datapath:
  bf16t — DEFAULT. bf16 operands, tap-outer order with ONE explicit
          LDWEIGHTS per (otile, tap) reused by the 7 row-chunk matmuls
          (7 PSUM banks live). Cuts the per-matmul 128-row weight-load
          tax from 28.6%% to 4%%. PSUM drains alternate between the
          Activation engine (activation+bias) and DVE (tensor_scalar_add
          +bias) so the end-of-otile drain burst is not single-engine
          bound. Weight DMA is split per otile and the first image's DMA
          is split into quarters across two queues to shorten the
          prologue.
  fp32r — chunk-serial fused-LDW fp32r path (old default, exact to 1e-4;
          each matmul re-streams 128 weight rows on HW: ~124us/rep).
  bf16  — chunk-serial fused-LDW bf16.
"""

import os as _os
from contextlib import ExitStack

import numpy as np

import concourse.bass as bass
import concourse.tile as tile
from concourse import bacc, mybir
from concourse.bass_utils import run_bass_kernel_spmd
from concourse.tile import add_dep_helper

N_CORES = 8
B_FULL, C, H, W = 32, 128, 56, 56
O = 256
KH = KW = 3
B = B_FULL // N_CORES          # images per core
PH, PW = H + 2, W + 2          # zero-padded image dims
CHUNK = 8                      # output rows per matmul group
NCHUNK = H // CHUNK            # 7
OTILES = O // 128              # 2
FDIM = CHUNK * W               # 448 moving elements per matmul

DT_MODE = _os.environ.get("K_DT", "bf16t")   # bf16t | fp32r | bf16
REPS = int(_os.environ.get("K_REPS", "1"))   # device-side repeat (timing)

PSUM_BUFS = int(_os.environ.get("K_PSUM_BUFS", "8"))
OUT_BUFS = int(_os.environ.get("K_OUT_BUFS", "6"))
PAD_BUFS = int(_os.environ.get("K_PAD_BUFS", "2"))

_CD = {"fp32r": mybir.dt.float32r, "bf16": mybir.dt.bfloat16,
       "bf16t": mybir.dt.bfloat16}
_BF16 = ("bf16", "bf16t")


def conv_body(ctx: ExitStack, tc: tile.TileContext, out: bass.AP, inp: bass.AP,
              ker: bass.AP, bias: bass.AP):
    """inp [B, C, PH, PW] pre-padded; ker [C, O, 9]; bias [128, OTILES];
    out [B, O, H, W]. inp/ker DRAM dtype: bf16 for bf16 modes else fp32."""
    nc = tc.nc
    cd = _CD[DT_MODE]
    bitcast = DT_MODE == "fp32r"   # DRAM fp32 bits reinterpreted as fp32r

    def as_cd(ap):
        return ap.bitcast(cd) if bitcast else ap

    singles = ctx.enter_context(tc.tile_pool(name="singles", bufs=1))
    psum_pool = ctx.enter_context(
        tc.tile_pool(name="psum", bufs=PSUM_BUFS, space="PSUM"))
    out_pool = ctx.enter_context(tc.tile_pool(name="outs", bufs=OUT_BUFS))

    # Weights [c, tap, o] (tap-major): tap 0 (both otiles, 64KB) ships in its
    # own first DMA so the first LDWEIGHTS waits on ~64KB, not the full set.
    # Scalar-engine queue keeps sync/gpsimd free for image 0's DMA.
    w_sb = singles.tile([C, KH * KW, O], cd)
    nc.scalar.dma_start(out=w_sb[:, 0, :], in_=as_cd(ker[:, 0, :]))
    nc.scalar.dma_start(out=w_sb[:, 1:, :], in_=as_cd(ker[:, 1:, :]))

    # Bias [p, otile]: bias for output channel ot*128+p.
    b_sb = singles.tile([128, OTILES], mybir.dt.float32)
    nc.gpsimd.dma_start(out=b_sb[:], in_=bias)

    # Padded image buffers, rotated across images; fully written by each DMA.
    pads = [singles.tile([C, PH, PW], cd, name=f"pad{i}", tag=f"pad{i}")
            for i in range(PAD_BUFS)]

    def drain(n, ot, chunk, ps):
        """PSUM->SBUF + bias, alternating Act/DVE so the end-of-otile burst is
        two-engine wide. All out-DMAs go on the sync (SP) queue: a DMACopy
        issue occupies its SEQ until HWDGE accepts it, so putting them on the
        scalar queue would stall the very drains they depend on."""
        y0 = chunk * CHUNK
        o_sb = out_pool.tile([128, FDIM], mybir.dt.float32, name="o_sb",
                             tag="o_sb")
        if DT_MODE == "bf16t" and chunk % 2 == 1:
            nc.vector.tensor_scalar_add(o_sb[:], ps[:], b_sb[:, ot:ot + 1])
        else:
            nc.scalar.activation(o_sb[:], ps[:],
                                 mybir.ActivationFunctionType.Identity,
                                 bias=b_sb[:, ot:ot + 1])
        o_eng = nc.sync if DT_MODE == "bf16t" else (
            nc.sync if (chunk % 2 == 0) else nc.scalar)
        o_eng.dma_start(out=out[n, ot * 128:(ot + 1) * 128, y0:y0 + CHUNK, :],
                        in_=o_sb[:])

    def rhs_ap(p_in, chunk, tap):
        dy, dx = tap // KW, tap % KW
        y0 = chunk * CHUNK
        return p_in[:, y0 + dy:y0 + dy + CHUNK, dx:dx + W]

    prev_pe = [None]

    def load_image(n, prologue=False):
        """Pad DMA for image n in row-quarters. Steady state: all on the
        gpsimd queue (its SWDGE path is separate from HWDGE, so inputs never
        contend with out-DMA issue). Prologue: split across sync+gpsimd for
        minimum latency before the first matmul."""
        p_in = pads[n % PAD_BUFS]
        i_src = as_cd(inp[n])
        q = PH // 4
        bounds = [0, q, 2 * q, 3 * q, PH]
        engs = ([nc.sync, nc.sync, nc.gpsimd, nc.gpsimd] if prologue
                else [nc.gpsimd] * 4)
        for e, lo, hi in zip(engs, bounds[:-1], bounds[1:]):
            e.dma_start(out=p_in[:, lo:hi, :], in_=i_src[:, lo:hi, :])
        return p_in

    def one_image_bf16t(n, p_in):
        for ot in range(OTILES):
            pss = [psum_pool.tile([128, FDIM], mybir.dt.float32,
                                  name="ps", tag="ps")
                   for _ in range(NCHUNK)]
            for tap in range(KH * KW):
                w_tap = w_sb[:, tap, ot * 128:(ot + 1) * 128]
                ldw = nc.tensor.ldweights(w_tap)
                if prev_pe[0] is not None:
                    add_dep_helper(ldw.ins, prev_pe[0].ins, False,
                                   "ldw after prev tap's matmuls")
                for chunk in range(NCHUNK):
                    mm = nc.tensor.matmul(
                        pss[chunk][:], w_tap,
                        rhs_ap(p_in, chunk, tap),
                        start=(tap == 0), stop=(tap == KH * KW - 1))
                    mm.ins.ldweights = False
                    add_dep_helper(mm.ins, ldw.ins, False,
                                   "matmul uses explicit ldweights")
                    prev_pe[0] = mm
            if ot == 0 and n + 1 < B:
                # Prefetch next image's input now: its WAW wait (pad reused
                # from image n-1) is already satisfied, and issuing here keeps
                # it clear of this image's tail on the gpsimd queue.
                load_image(n + 1)
            for chunk in range(NCHUNK):
                drain(n, ot, chunk, pss[chunk])

    def one_image_fused(n):
        p_in = load_image(n, prologue=(n == 0))
        for ot in range(OTILES):
            for chunk in range(NCHUNK):
                ps = psum_pool.tile([128, FDIM], mybir.dt.float32,
                                    name="ps", tag="ps")
                for tap in range(KH * KW):
                    nc.tensor.matmul(ps[:], w_sb[:, tap,
                                             ot * 128:(ot + 1) * 128],
                                     rhs_ap(p_in, chunk, tap),
                                     start=(tap == 0),
                                     stop=(tap == KH * KW - 1))
                drain(n, ot, chunk, ps)

    def body():
        if DT_MODE == "bf16t":
            p_in = load_image(0, prologue=True)
            for n in range(B):
                nxt = pads[(n + 1) % PAD_BUFS]
                one_image_bf16t(n, p_in)
                p_in = nxt
        else:
            for n in range(B):
                one_image_fused(n)

    reps = getattr(tc, "_k_reps", REPS)
    if reps > 1:
        with tc.For_i(0, reps, 1):
            body()
    else:
        body()


def build_nc(reps: int | None = None) -> bass.Bass:
    in_dt = _CD[DT_MODE] if DT_MODE in _BF16 else mybir.dt.float32
    nc = bacc.Bacc(trn_type="TRN2", target_bir_lowering=False, debug=False)
    inp = nc.dram_tensor("inp", [B, C, PH, PW], in_dt,
                         kind="ExternalInput").ap()
    ker = nc.dram_tensor("kernel", [C, KH * KW, O], in_dt,
                         kind="ExternalInput").ap()
    bias = nc.dram_tensor("bias", [128, OTILES], mybir.dt.float32,
                          kind="ExternalInput").ap()
    out = nc.dram_tensor("out", [B, O, H, W], mybir.dt.float32,
                         kind="ExternalOutput").ap()
    with tile.TileContext(nc) as tc:
        if reps is not None:
            tc._k_reps = reps
        with ExitStack() as ctx:
            conv_body(ctx, tc, out, inp, ker, bias)
    nc.compile()
    return nc


def host_prep(inp, kernel, bias):
    """Shard-side layout prep: pad + transpose + cast to the DRAM dtypes."""
    inp = np.ascontiguousarray(inp, dtype=np.float32)
    kernel = np.ascontiguousarray(kernel, dtype=np.float32)
    bias = np.ascontiguousarray(bias, dtype=np.float32)
    if DT_MODE in _BF16:
        import ml_dtypes
        np_dt = ml_dtypes.bfloat16
    else:
        np_dt = np.float32
    inp_pad = np.zeros((B_FULL, C, PH, PW), np_dt)
    inp_pad[:, :, 1:1 + H, 1:1 + W] = inp
    # [O, C, kh, kw] -> [C, kh*kw, O] (tap-major)
    w_host = np.ascontiguousarray(
        kernel.reshape(O, C, KH * KW).transpose(1, 2, 0)).astype(np_dt)
    b_host = np.ascontiguousarray(bias.reshape(OTILES, 128).T)
    return inp_pad, w_host, b_host


_NC_CACHE = None


def kernel(inp: np.ndarray, kernel: np.ndarray, bias: np.ndarray) -> np.ndarray:
    global _NC_CACHE
    if _NC_CACHE is None:
        _NC_CACHE = build_nc()
    nc = _NC_CACHE
    inp_pad, w_host, b_host = host_prep(inp, kernel, bias)
    in_maps = [
        {"inp": inp_pad[i * B:(i + 1) * B], "kernel": w_host, "bias": b_host}
        for i in range(N_CORES)
    ]
    res = run_bass_kernel_spmd(nc, in_maps, core_ids=list(range(N_CORES)))
    return np.concatenate([r["out"] for r in res.results], axis=0)


# revision 27
# speedup vs baseline: 1.0535x; 1.0535x over previous
"""Trainium2 Bass kernel: 3x3 SAME conv (NCHW/OIHW) + bias.

Full problem: inp (32,128,56,56) f32, kernel (256,128,3,3) f32, bias (256,) f32
-> out (32,256,56,56) f32.

Strategy: data-parallel over batch across 8 cores (4 images/core). Host-side
prep inside kernel(): zero-pad images to [128,58,58], transpose weights to
[C,O,9], reshape bias to [128,2] — every device DMA is contiguous. Per image,
implicit GEMM: contraction K = C_in = 128 on the partition dim, M = 128 output
channels per tile (2 tiles), moving N = 448 spatial pixels (8 output rows x 56
cols via a strided AP over the padded SBUF image).

DT_MODE selects the PE datapath:
  bf16t — DEFAULT. bf16 operands, tap-outer order with ONE explicit
          LDWEIGHTS per (otile, tap) reused by the 7 row-chunk matmuls
          (7 PSUM banks live). Cuts the per-matmul 128-row weight-load
          tax from 28.6%% to 4%%. PSUM drains alternate between the
          Activation engine (activation+bias) and DVE (tensor_scalar_add
          +bias) so the end-of-otile drain burst is not single-engine
          bound. Weight DMA is split per otile and the first image's DMA
          is split into quarters across two queues to shorten the
          prologue.
  fp32r — chunk-serial fused-LDW fp32r path (old default, exact to 1e-4;
          each matmul re-streams 128 weight rows on HW: ~124us/rep).
  bf16  — chunk-serial fused-LDW bf16.
"""

import os as _os
from contextlib import ExitStack

import numpy as np

import concourse.bass as bass
import concourse.tile as tile
from concourse import bacc, mybir
from concourse.bass_utils import run_bass_kernel_spmd
from concourse.tile import add_dep_helper

N_CORES = 8
B_FULL, C, H, W = 32, 128, 56, 56
O = 256
KH = KW = 3
B = B_FULL // N_CORES          # images per core
PH, PW = H + 2, W + 2          # zero-padded image dims
CHUNK = 8                      # output rows per matmul group
NCHUNK = H // CHUNK            # 7
OTILES = O // 128              # 2
FDIM = CHUNK * W               # 448 moving elements per matmul

DT_MODE = _os.environ.get("K_DT", "bf16t")   # bf16t | fp32r | bf16
REPS = int(_os.environ.get("K_REPS", "1"))   # device-side repeat (timing)

PSUM_BUFS = int(_os.environ.get("K_PSUM_BUFS", "8"))
OUT_BUFS = int(_os.environ.get("K_OUT_BUFS", "6"))
PAD_BUFS = int(_os.environ.get("K_PAD_BUFS", "4"))
# gpsimd as 3rd drain engine — rejected by the BIR verifier ("GPSIMD
# Instructions cannot access PSUM"); kept only as an experiment flag.
DRAIN3 = _os.environ.get("K_DRAIN3", "0") == "1"

_CD = {"fp32r": mybir.dt.float32r, "bf16": mybir.dt.bfloat16,
       "bf16t": mybir.dt.bfloat16}
_BF16 = ("bf16", "bf16t")


def conv_body(ctx: ExitStack, tc: tile.TileContext, out: bass.AP, inp: bass.AP,
              ker: bass.AP, bias: bass.AP):
    """inp [B, C, PH, PW] pre-padded; ker [C, O, 9]; bias [128, OTILES];
    out [B, O, H, W]. inp/ker DRAM dtype: bf16 for bf16 modes else fp32."""
    nc = tc.nc
    cd = _CD[DT_MODE]
    bitcast = DT_MODE == "fp32r"   # DRAM fp32 bits reinterpreted as fp32r

    def as_cd(ap):
        return ap.bitcast(cd) if bitcast else ap

    singles = ctx.enter_context(tc.tile_pool(name="singles", bufs=1))
    psum_pool = ctx.enter_context(
        tc.tile_pool(name="psum", bufs=PSUM_BUFS, space="PSUM"))
    out_pool = ctx.enter_context(tc.tile_pool(name="outs", bufs=OUT_BUFS))

    # Padded image buffers, rotated across images; fully written by each DMA.
    pads = [singles.tile([C, PH, PW], cd, name=f"pad{i}", tag=f"pad{i}")
            for i in range(PAD_BUFS)]

    def load_image(n, prologue=False):
        """Pad DMA for image n in row-quarters. Steady state: all on the
        gpsimd queue (its SWDGE path is separate from HWDGE) unless gpsimd is
        a drain engine, then sync. Prologue: spread across all three queues —
        the first-tap matmuls read rows top-down, so early quarters must hit
        the DMA pipe first."""
        p_in = pads[n % PAD_BUFS]
        i_src = as_cd(inp[n])
        q = PH // 4
        bounds = [0, q, 2 * q, 3 * q, PH]
        engs = ([nc.scalar, nc.sync, nc.sync, nc.gpsimd] if prologue
                else ([nc.sync] * 4 if DRAIN3 else [nc.gpsimd] * 4))
        for e, lo, hi in zip(engs, bounds[:-1], bounds[1:]):
            e.dma_start(out=p_in[:, lo:hi, :], in_=i_src[:, lo:hi, :])
        return p_in

    # Weights [c, tap, o] (tap-major), one DMA per tap: the first LDWEIGHTS
    # waits on a single 64KB tap. Tap 0 is issued before image 0's quarters,
    # the rest after, so the DMA pipe delivers exactly what the first
    # matmuls need first: w0, q1, q2, ...
    w_sb = singles.tile([C, KH * KW, O], cd)
    nc.scalar.dma_start(out=w_sb[:, 0, :], in_=as_cd(ker[:, 0, :]))

    # Image 0 loads once, outside the reps loop (with PAD_BUFS=4 each image
    # owns a pad, so pad 0 stays valid across reps and is never reloaded).
    load_image(0, prologue=True)

    for tap in range(1, KH * KW):
        nc.scalar.dma_start(out=w_sb[:, tap, :], in_=as_cd(ker[:, tap, :]))

    # Bias [p, otile]: bias for output channel ot*128+p.
    b_sb = singles.tile([128, OTILES], mybir.dt.float32)
    nc.gpsimd.dma_start(out=b_sb[:], in_=bias)

    def drain(n, ot, chunk, ps):
        """PSUM->SBUF + bias, alternating Act/DVE so the end-of-otile burst is
        two-engine wide. All out-DMAs go on the sync (SP) queue: a DMACopy
        issue occupies its SEQ until HWDGE accepts it, so putting them on the
        scalar queue would stall the very drains they depend on."""
        y0 = chunk * CHUNK
        o_sb = out_pool.tile([128, FDIM], mybir.dt.float32, name="o_sb",
                             tag="o_sb")
        if DRAIN3 and DT_MODE == "bf16t" and chunk % 3 == 2:
            nc.gpsimd.tensor_scalar_add(o_sb[:], ps[:], b_sb[:, ot:ot + 1])
        elif DT_MODE == "bf16t" and (chunk % 3 == 1 if DRAIN3
                                     else chunk % 2 == 1):
            nc.vector.tensor_scalar_add(o_sb[:], ps[:], b_sb[:, ot:ot + 1])
        else:
            nc.scalar.activation(o_sb[:], ps[:],
                                 mybir.ActivationFunctionType.Identity,
                                 bias=b_sb[:, ot:ot + 1])
        o_eng = nc.sync if DT_MODE == "bf16t" else (
            nc.sync if (chunk % 2 == 0) else nc.scalar)
        o_eng.dma_start(out=out[n, ot * 128:(ot + 1) * 128, y0:y0 + CHUNK, :],
                        in_=o_sb[:])

    def rhs_ap(p_in, chunk, tap):
        dy, dx = tap // KW, tap % KW
        y0 = chunk * CHUNK
        return p_in[:, y0 + dy:y0 + dy + CHUNK, dx:dx + W]

    prev_pe = [None]

    def one_image_bf16t(n, p_in):
        for ot in range(OTILES):
            if n == B - 1 and ot == OTILES - 1:
                # Final otile: two tap-outer waves (chunks 0-4, then 5-6).
                # Wave 1's drains+out-DMAs pipeline behind wave 2's matmuls,
                # so the end-of-body barrier (For_i) / kernel tail only waits
                # on 2 drains instead of a 7-deep burst. Costs one extra set
                # of 9 LDWs.
                for wave in (range(0, 5), range(5, NCHUNK)):
                    wave = list(wave)
                    pss = {c: psum_pool.tile([128, FDIM], mybir.dt.float32,
                                             name="ps", tag="ps")
                           for c in wave}
                    for tap in range(KH * KW):
                        w_tap = w_sb[:, tap, ot * 128:(ot + 1) * 128]
                        ldw = nc.tensor.ldweights(w_tap)
                        add_dep_helper(ldw.ins, prev_pe[0].ins, False,
                                       "ldw after prev tap's matmuls")
                        for chunk in wave:
                            mm = nc.tensor.matmul(
                                pss[chunk][:], w_tap,
                                rhs_ap(p_in, chunk, tap),
                                start=(tap == 0), stop=(tap == KH * KW - 1))
                            mm.ins.ldweights = False
                            add_dep_helper(mm.ins, ldw.ins, False,
                                           "matmul uses explicit ldweights")
                            prev_pe[0] = mm
                    for chunk in wave:
                        drain(n, ot, chunk, pss[chunk])
                continue
            pss = [psum_pool.tile([128, FDIM], mybir.dt.float32,
                                  name="ps", tag="ps")
                   for _ in range(NCHUNK)]
            for tap in range(KH * KW):
                w_tap = w_sb[:, tap, ot * 128:(ot + 1) * 128]
                ldw = nc.tensor.ldweights(w_tap)
                if prev_pe[0] is not None:
                    add_dep_helper(ldw.ins, prev_pe[0].ins, False,
                                   "ldw after prev tap's matmuls")
                for chunk in range(NCHUNK):
                    mm = nc.tensor.matmul(
                        pss[chunk][:], w_tap,
                        rhs_ap(p_in, chunk, tap),
                        start=(tap == 0), stop=(tap == KH * KW - 1))
                    mm.ins.ldweights = False
                    add_dep_helper(mm.ins, ldw.ins, False,
                                   "matmul uses explicit ldweights")
                    prev_pe[0] = mm
            if ot == 0 and n + 1 < B:
                # Prefetch the next image's input now. With one pad per
                # image there is no WAW wait at all; image 0 is loop-
                # invariant and never reloaded.
                load_image(n + 1)
            for chunk in range(NCHUNK):
                drain(n, ot, chunk, pss[chunk])

    def one_image_fused(n):
        p_in = pads[n % PAD_BUFS] if n == 0 else load_image(n)
        for ot in range(OTILES):
            for chunk in range(NCHUNK):
                ps = psum_pool.tile([128, FDIM], mybir.dt.float32,
                                    name="ps", tag="ps")
                for tap in range(KH * KW):
                    nc.tensor.matmul(ps[:], w_sb[:, tap,
                                             ot * 128:(ot + 1) * 128],
                                     rhs_ap(p_in, chunk, tap),
                                     start=(tap == 0),
                                     stop=(tap == KH * KW - 1))
                drain(n, ot, chunk, ps)

    def body():
        if DT_MODE == "bf16t":
            for n in range(B):
                one_image_bf16t(n, pads[n % PAD_BUFS])
        else:
            for n in range(B):
                one_image_fused(n)

    reps = getattr(tc, "_k_reps", REPS)
    if reps > 1:
        with tc.For_i(0, reps, 1):
            body()
    else:
        body()


def build_nc(reps: int | None = None) -> bass.Bass:
    in_dt = _CD[DT_MODE] if DT_MODE in _BF16 else mybir.dt.float32
    nc = bacc.Bacc(trn_type="TRN2", target_bir_lowering=False, debug=False)
    inp = nc.dram_tensor("inp", [B, C, PH, PW], in_dt,
                         kind="ExternalInput").ap()
    ker = nc.dram_tensor("kernel", [C, KH * KW, O], in_dt,
                         kind="ExternalInput").ap()
    bias = nc.dram_tensor("bias", [128, OTILES], mybir.dt.float32,
                          kind="ExternalInput").ap()
    out = nc.dram_tensor("out", [B, O, H, W], mybir.dt.float32,
                         kind="ExternalOutput").ap()
    with tile.TileContext(nc) as tc:
        if reps is not None:
            tc._k_reps = reps
        with ExitStack() as ctx:
            conv_body(ctx, tc, out, inp, ker, bias)
    nc.compile()
    return nc


def host_prep(inp, kernel, bias):
    """Shard-side layout prep: pad + transpose + cast to the DRAM dtypes."""
    inp = np.ascontiguousarray(inp, dtype=np.float32)
    kernel = np.ascontiguousarray(kernel, dtype=np.float32)
    bias = np.ascontiguousarray(bias, dtype=np.float32)
    if DT_MODE in _BF16:
        import ml_dtypes
        np_dt = ml_dtypes.bfloat16
    else:
        np_dt = np.float32
    inp_pad = np.zeros((B_FULL, C, PH, PW), np_dt)
    inp_pad[:, :, 1:1 + H, 1:1 + W] = inp
    # [O, C, kh, kw] -> [C, kh*kw, O] (tap-major)
    w_host = np.ascontiguousarray(
        kernel.reshape(O, C, KH * KW).transpose(1, 2, 0)).astype(np_dt)
    b_host = np.ascontiguousarray(bias.reshape(OTILES, 128).T)
    return inp_pad, w_host, b_host


_NC_CACHE = None


def kernel(inp: np.ndarray, kernel: np.ndarray, bias: np.ndarray) -> np.ndarray:
    global _NC_CACHE
    if _NC_CACHE is None:
        _NC_CACHE = build_nc()
    nc = _NC_CACHE
    inp_pad, w_host, b_host = host_prep(inp, kernel, bias)
    in_maps = [
        {"inp": inp_pad[i * B:(i + 1) * B], "kernel": w_host, "bias": b_host}
        for i in range(N_CORES)
    ]
    res = run_bass_kernel_spmd(nc, in_maps, core_ids=list(range(N_CORES)))
    return np.concatenate([r["out"] for r in res.results], axis=0)


# revision 29
# speedup vs baseline: 1.0713x; 1.0169x over previous
"""Trainium2 Bass kernel: 3x3 SAME conv (NCHW/OIHW) + bias.

Full problem: inp (32,128,56,56) f32, kernel (256,128,3,3) f32, bias (256,) f32
-> out (32,256,56,56) f32.

Strategy: data-parallel over batch across 8 cores (4 images/core). Host-side
prep inside kernel(): zero-pad images to [128,58,58] bf16, transpose weights
to tap-major [C,9,O] bf16, reshape bias to [128,2] f32 — every device DMA is
contiguous. Per image, implicit GEMM: contraction K = C_in = 128 on the
partition dim, M = 128 output channels per tile (2 otiles), moving N = 448
spatial pixels (8 output rows x 56 cols via a strided AP over the padded SBUF
image); PSUM accumulates the 9 taps.

DT_MODE selects the PE datapath:
  bf16t — DEFAULT (~106us/rep HW steady state, PE-bound; roofline 94us).
          Tap-outer order with ONE explicit LDWEIGHTS per (otile, tap)
          reused by the 7 row-chunk matmuls (7 PSUM banks live) — cuts the
          per-matmul 128-row weight-load tax from 28.6%% to 4%%. Engine/
          queue discipline, each found as a stall in the timeline trace:
          * PSUM drains alternate Activation / DVE (two-engine drain burst);
          * out-DMAs ride the sync (SP) queue only — a DMACopy issue parks
            on its SEQ until HWDGE accepts it, so putting them on the
            scalar queue would stall the drains behind them;
          * input images ride the gpsimd queue (SWDGE, separate path), one
            pad buffer per image, prefetched an image ahead, image 0 loaded
            once outside the reps loop;
          * weights ship one 64KB tap per DMA, tap 0 ahead of image 0's
            quarters, so the first LDWEIGHTS+matmuls start ~2us in;
          * the last image's second otile runs in tap-outer waves (4/2/1
            chunks) so the final drain+DMA burst pipelines behind compute
            and the For_i end-of-body barrier waits on one chunk, not 7.
  fp32r — chunk-serial fused-LDW fp32r path (exact to 1e-4; each matmul
          re-streams 128 weight rows on HW: ~124us/rep).
  bf16  — chunk-serial fused-LDW bf16.
"""

import os as _os
from contextlib import ExitStack

import numpy as np

import concourse.bass as bass
import concourse.tile as tile
from concourse import bacc, mybir
from concourse.bass_utils import run_bass_kernel_spmd
from concourse.tile import add_dep_helper

N_CORES = 8
B_FULL, C, H, W = 32, 128, 56, 56
O = 256
KH = KW = 3
B = B_FULL // N_CORES          # images per core
PH, PW = H + 2, W + 2          # zero-padded image dims
CHUNK = 8                      # output rows per matmul group
NCHUNK = H // CHUNK            # 7
OTILES = O // 128              # 2
FDIM = CHUNK * W               # 448 moving elements per matmul

DT_MODE = _os.environ.get("K_DT", "bf16t")   # bf16t | fp32r | bf16
REPS = int(_os.environ.get("K_REPS", "1"))   # device-side repeat (timing)

PSUM_BUFS = int(_os.environ.get("K_PSUM_BUFS", "8"))
OUT_BUFS = int(_os.environ.get("K_OUT_BUFS", "6"))
PAD_BUFS = int(_os.environ.get("K_PAD_BUFS", "4"))
# gpsimd as 3rd drain engine — rejected by the BIR verifier ("GPSIMD
# Instructions cannot access PSUM"); kept only as an experiment flag.
DRAIN3 = _os.environ.get("K_DRAIN3", "0") == "1"

_CD = {"fp32r": mybir.dt.float32r, "bf16": mybir.dt.bfloat16,
       "bf16t": mybir.dt.bfloat16}
_BF16 = ("bf16", "bf16t")


def conv_body(ctx: ExitStack, tc: tile.TileContext, out: bass.AP, inp: bass.AP,
              ker: bass.AP, bias: bass.AP):
    """inp [B, C, PH, PW] pre-padded; ker [C, O, 9]; bias [128, OTILES];
    out [B, O, H, W]. inp/ker DRAM dtype: bf16 for bf16 modes else fp32."""
    nc = tc.nc
    cd = _CD[DT_MODE]
    bitcast = DT_MODE == "fp32r"   # DRAM fp32 bits reinterpreted as fp32r

    def as_cd(ap):
        return ap.bitcast(cd) if bitcast else ap

    singles = ctx.enter_context(tc.tile_pool(name="singles", bufs=1))
    psum_pool = ctx.enter_context(
        tc.tile_pool(name="psum", bufs=PSUM_BUFS, space="PSUM"))
    out_pool = ctx.enter_context(tc.tile_pool(name="outs", bufs=OUT_BUFS))

    # Padded image buffers, rotated across images; fully written by each DMA.
    pads = [singles.tile([C, PH, PW], cd, name=f"pad{i}", tag=f"pad{i}")
            for i in range(PAD_BUFS)]

    def load_image(n, prologue=False):
        """Pad DMA for image n in row-quarters. Steady state: all on the
        gpsimd queue (its SWDGE path is separate from HWDGE) unless gpsimd is
        a drain engine, then sync. Prologue: spread across all three queues —
        the first-tap matmuls read rows top-down, so early quarters must hit
        the DMA pipe first."""
        p_in = pads[n % PAD_BUFS]
        i_src = as_cd(inp[n])
        q = PH // 4
        bounds = [0, q, 2 * q, 3 * q, PH]
        engs = ([nc.scalar, nc.sync, nc.sync, nc.gpsimd] if prologue
                else ([nc.sync] * 4 if DRAIN3 else [nc.gpsimd] * 4))
        for e, lo, hi in zip(engs, bounds[:-1], bounds[1:]):
            e.dma_start(out=p_in[:, lo:hi, :], in_=i_src[:, lo:hi, :])
        return p_in

    # Weights [c, tap, o] (tap-major), one DMA per tap: the first LDWEIGHTS
    # waits on a single 64KB tap. Tap 0 is issued before image 0's quarters,
    # the rest after, so the DMA pipe delivers exactly what the first
    # matmuls need first: w0, q1, q2, ...
    w_sb = singles.tile([C, KH * KW, O], cd)
    nc.scalar.dma_start(out=w_sb[:, 0, :], in_=as_cd(ker[:, 0, :]))

    # Image 0 loads once, outside the reps loop (with PAD_BUFS=4 each image
    # owns a pad, so pad 0 stays valid across reps and is never reloaded).
    load_image(0, prologue=True)

    for tap in range(1, KH * KW):
        nc.scalar.dma_start(out=w_sb[:, tap, :], in_=as_cd(ker[:, tap, :]))

    # Bias [p, otile]: bias for output channel ot*128+p.
    b_sb = singles.tile([128, OTILES], mybir.dt.float32)
    nc.gpsimd.dma_start(out=b_sb[:], in_=bias)

    def drain(n, ot, chunk, ps):
        """PSUM->SBUF + bias, alternating Act/DVE so the end-of-otile burst is
        two-engine wide. All out-DMAs go on the sync (SP) queue: a DMACopy
        issue occupies its SEQ until HWDGE accepts it, so putting them on the
        scalar queue would stall the very drains they depend on."""
        y0 = chunk * CHUNK
        o_sb = out_pool.tile([128, FDIM], mybir.dt.float32, name="o_sb",
                             tag="o_sb")
        if DRAIN3 and DT_MODE == "bf16t" and chunk % 3 == 2:
            nc.gpsimd.tensor_scalar_add(o_sb[:], ps[:], b_sb[:, ot:ot + 1])
        elif DT_MODE == "bf16t" and (chunk % 3 == 1 if DRAIN3
                                     else chunk % 2 == 1):
            nc.vector.tensor_scalar_add(o_sb[:], ps[:], b_sb[:, ot:ot + 1])
        else:
            nc.scalar.activation(o_sb[:], ps[:],
                                 mybir.ActivationFunctionType.Identity,
                                 bias=b_sb[:, ot:ot + 1])
        o_eng = nc.sync if DT_MODE == "bf16t" else (
            nc.sync if (chunk % 2 == 0) else nc.scalar)
        o_eng.dma_start(out=out[n, ot * 128:(ot + 1) * 128, y0:y0 + CHUNK, :],
                        in_=o_sb[:])

    def rhs_ap(p_in, chunk, tap):
        dy, dx = tap // KW, tap % KW
        y0 = chunk * CHUNK
        return p_in[:, y0 + dy:y0 + dy + CHUNK, dx:dx + W]

    prev_pe = [None]

    def one_image_bf16t(n, p_in):
        for ot in range(OTILES):
            if n == B - 1 and ot == OTILES - 1:
                # Final otile: tap-outer in waves (chunks 0-3, 4-5, then 6).
                # Earlier waves' drains+out-DMAs pipeline behind later waves'
                # matmuls, so the end-of-body barrier (For_i) / kernel tail
                # only waits on the last chunk's drain+DMA instead of a
                # 7-deep burst. Costs two extra sets of 9 LDWs.
                for wave in (range(0, 4), range(4, 6), range(6, NCHUNK)):
                    wave = list(wave)
                    pss = {c: psum_pool.tile([128, FDIM], mybir.dt.float32,
                                             name="ps", tag="ps")
                           for c in wave}
                    for tap in range(KH * KW):
                        w_tap = w_sb[:, tap, ot * 128:(ot + 1) * 128]
                        ldw = nc.tensor.ldweights(w_tap)
                        add_dep_helper(ldw.ins, prev_pe[0].ins, False,
                                       "ldw after prev tap's matmuls")
                        for chunk in wave:
                            mm = nc.tensor.matmul(
                                pss[chunk][:], w_tap,
                                rhs_ap(p_in, chunk, tap),
                                start=(tap == 0), stop=(tap == KH * KW - 1))
                            mm.ins.ldweights = False
                            add_dep_helper(mm.ins, ldw.ins, False,
                                           "matmul uses explicit ldweights")
                            prev_pe[0] = mm
                    for chunk in wave:
                        drain(n, ot, chunk, pss[chunk])
                continue
            pss = [psum_pool.tile([128, FDIM], mybir.dt.float32,
                                  name="ps", tag="ps")
                   for _ in range(NCHUNK)]
            for tap in range(KH * KW):
                w_tap = w_sb[:, tap, ot * 128:(ot + 1) * 128]
                ldw = nc.tensor.ldweights(w_tap)
                if prev_pe[0] is not None:
                    add_dep_helper(ldw.ins, prev_pe[0].ins, False,
                                   "ldw after prev tap's matmuls")
                for chunk in range(NCHUNK):
                    mm = nc.tensor.matmul(
                        pss[chunk][:], w_tap,
                        rhs_ap(p_in, chunk, tap),
                        start=(tap == 0), stop=(tap == KH * KW - 1))
                    mm.ins.ldweights = False
                    add_dep_helper(mm.ins, ldw.ins, False,
                                   "matmul uses explicit ldweights")
                    prev_pe[0] = mm
            if ot == 0 and n + 1 < B:
                # Prefetch the next image's input now. With one pad per
                # image there is no WAW wait at all; image 0 is loop-
                # invariant and never reloaded.
                load_image(n + 1)
            for chunk in range(NCHUNK):
                drain(n, ot, chunk, pss[chunk])

    def one_image_fused(n):
        p_in = pads[n % PAD_BUFS] if n == 0 else load_image(n)
        for ot in range(OTILES):
            for chunk in range(NCHUNK):
                ps = psum_pool.tile([128, FDIM], mybir.dt.float32,
                                    name="ps", tag="ps")
                for tap in range(KH * KW):
                    nc.tensor.matmul(ps[:], w_sb[:, tap,
                                             ot * 128:(ot + 1) * 128],
                                     rhs_ap(p_in, chunk, tap),
                                     start=(tap == 0),
                                     stop=(tap == KH * KW - 1))
                drain(n, ot, chunk, ps)

    def body():
        if DT_MODE == "bf16t":
            for n in range(B):
                one_image_bf16t(n, pads[n % PAD_BUFS])
        else:
            for n in range(B):
                one_image_fused(n)

    reps = getattr(tc, "_k_reps", REPS)
    if reps > 1:
        with tc.For_i(0, reps, 1):
            body()
    else:
        body()


def build_nc(reps: int | None = None) -> bass.Bass:
    in_dt = _CD[DT_MODE] if DT_MODE in _BF16 else mybir.dt.float32
    nc = bacc.Bacc(trn_type="TRN2", target_bir_lowering=False, debug=False)
    inp = nc.dram_tensor("inp", [B, C, PH, PW], in_dt,
                         kind="ExternalInput").ap()
    ker = nc.dram_tensor("kernel", [C, KH * KW, O], in_dt,
                         kind="ExternalInput").ap()
    bias = nc.dram_tensor("bias", [128, OTILES], mybir.dt.float32,
                          kind="ExternalInput").ap()
    out = nc.dram_tensor("out", [B, O, H, W], mybir.dt.float32,
                         kind="ExternalOutput").ap()
    with tile.TileContext(nc) as tc:
        if reps is not None:
            tc._k_reps = reps
        with ExitStack() as ctx:
            conv_body(ctx, tc, out, inp, ker, bias)
    nc.compile()
    return nc


def host_prep(inp, kernel, bias):
    """Shard-side layout prep: pad + transpose + cast to the DRAM dtypes."""
    inp = np.ascontiguousarray(inp, dtype=np.float32)
    kernel = np.ascontiguousarray(kernel, dtype=np.float32)
    bias = np.ascontiguousarray(bias, dtype=np.float32)
    if DT_MODE in _BF16:
        import ml_dtypes
        np_dt = ml_dtypes.bfloat16
    else:
        np_dt = np.float32
    inp_pad = np.zeros((B_FULL, C, PH, PW), np_dt)
    inp_pad[:, :, 1:1 + H, 1:1 + W] = inp
    # [O, C, kh, kw] -> [C, kh*kw, O] (tap-major)
    w_host = np.ascontiguousarray(
        kernel.reshape(O, C, KH * KW).transpose(1, 2, 0)).astype(np_dt)
    b_host = np.ascontiguousarray(bias.reshape(OTILES, 128).T)
    return inp_pad, w_host, b_host


_NC_CACHE = None


def kernel(inp: np.ndarray, kernel: np.ndarray, bias: np.ndarray) -> np.ndarray:
    global _NC_CACHE
    if _NC_CACHE is None:
        _NC_CACHE = build_nc()
    nc = _NC_CACHE
    inp_pad, w_host, b_host = host_prep(inp, kernel, bias)
    in_maps = [
        {"inp": inp_pad[i * B:(i + 1) * B], "kernel": w_host, "bias": b_host}
        for i in range(N_CORES)
    ]
    res = run_bass_kernel_spmd(nc, in_maps, core_ids=list(range(N_CORES)))
    return np.concatenate([r["out"] for r in res.results], axis=0)
